# revision 11
# baseline (speedup 1.0000x reference)
"""Distributed Bass kernel for nn_AttentionLayer (B=2, S=2048, H=1024, NH=16).

Sharding: 8 cores = 2 batch groups x 4 ranks. Core c handles batch c//4 and
heads [4r:4r+4] (r = c%4). QKV projections are column-sharded, attention runs
per-head with a transposed dataflow (scores^T so softmax's reduction axis sits
on PSUM partitions and feeds the ctx matmul directly), the output projection
is row-sharded, and a 4-rank ReduceScatter combines the partial outputs.
Residual + LayerNorm run on each rank's token shard.

All TensorEngine traffic is bf16 (fp32 matmul is 4x slower); accumulation
stays fp32 in PSUM. Softmax skips the max-subtraction (scores ~ N(0,1)) and
defers normalization: a ones-column appended to V yields the exp-sums as a
65th PSUM row of the ctx matmul.
"""

import sys
from contextlib import ExitStack

sys.path.insert(0, "/opt/trn_rl_repo")

import numpy as np
from concourse import bacc, bass, bass_utils, mybir, tile
from concourse.masks import make_identity

AF = mybir.ActivationFunctionType
ALU = mybir.AluOpType
F32 = mybir.dt.float32
BF16 = mybir.dt.bfloat16

B, S, H, NH, HD = 2, 2048, 1024, 16, 64
N_CORES = 8
RANKS = 4  # ranks per batch group
GROUPS = [[0, 1, 2, 3], [4, 5, 6, 7]]
HPC = NH // RANKS  # heads per core = 4
DLOC = HPC * HD  # local head dims = 256
SSH = S // RANKS  # token shard = 512
LN_EPS = 1e-5
P = 128
KO = H // P  # 8 k-tiles over hidden dim
TI = S // P  # 16 token tiles
QH = 1024  # q half width
NKC = S // P  # 16 key chunks


def build():
    nc = bacc.Bacc("TRN2", target_bir_lowering=False, debug=False, num_devices=N_CORES)

    xb = nc.dram_tensor("xb", [S, H], F32, kind="ExternalInput")
    xres = nc.dram_tensor("xres", [SSH, H], F32, kind="ExternalInput")
    wq_d = nc.dram_tensor("wq", [H, DLOC], F32, kind="ExternalInput")
    wk_d = nc.dram_tensor("wk", [H, DLOC], F32, kind="ExternalInput")
    wv_d = nc.dram_tensor("wv", [H, DLOC], F32, kind="ExternalInput")
    wo_d = nc.dram_tensor("wo", [DLOC, H], F32, kind="ExternalInput")
    bq_d = nc.dram_tensor("bq", [DLOC], F32, kind="ExternalInput")
    bk_d = nc.dram_tensor("bk", [DLOC], F32, kind="ExternalInput")
    bv_d = nc.dram_tensor("bv", [DLOC], F32, kind="ExternalInput")
    bo_d = nc.dram_tensor("bo", [H], F32, kind="ExternalInput")
    gamma_d = nc.dram_tensor("gamma", [H], F32, kind="ExternalInput")
    beta_d = nc.dram_tensor("beta", [H], F32, kind="ExternalInput")
    out_d = nc.dram_tensor("out", [SSH, H], F32, kind="ExternalOutput")

    with tile.TileContext(nc) as tc, ExitStack() as ctx:
        _build_body(
            nc, tc, ctx,
            xb, xres, wq_d, wk_d, wv_d, wo_d, bq_d, bk_d, bv_d, bo_d,
            gamma_d, beta_d, out_d,
        )
    return nc


def _build_body(
    nc, tc, ctx, xb, xres, wq_d, wk_d, wv_d, wo_d, bq_d, bk_d, bv_d, bo_d,
    gamma_d, beta_d, out_d,
):
    const = ctx.enter_context(tc.tile_pool(name="const", bufs=1))
    stg = ctx.enter_context(tc.tile_pool(name="stg", bufs=3))
    work = ctx.enter_context(tc.tile_pool(name="work", bufs=2))
    expp = ctx.enter_context(tc.tile_pool(name="expp", bufs=4))
    small = ctx.enter_context(tc.tile_pool(name="small", bufs=2))
    epi = ctx.enter_context(tc.tile_pool(name="epi", bufs=2))
    dram = ctx.enter_context(tc.tile_pool(name="dram", bufs=1, space="DRAM"))
    psA = ctx.enter_context(tc.tile_pool(name="psA", bufs=2, space="PSUM"))
    psB = ctx.enter_context(tc.tile_pool(name="psB", bufs=2, space="PSUM"))

    partial_d = dram.tile([S, H], BF16)
    rs_d = dram.tile([SSH, H], BF16)

    # ---- constants / weights ----
    ident = const.tile([P, P], BF16)
    make_identity(nc, ident[:])

    def load_w(dram_t, ko_n, free_n):
        # [ko_n*P, free_n] fp32 DRAM -> [P, ko_n, free_n] bf16 SBUF
        sb = const.tile([P, ko_n, free_n], BF16, tag=f"w_{dram_t.name}")
        for k2 in range(0, ko_n, 2):
            st = stg.tile([P, 2, free_n], F32, tag="wstg")
            nc.sync.dma_start(
                st[:], dram_t[:].rearrange("(ko p) f -> p ko f", p=P)[:, k2 : k2 + 2]
            )
            nc.vector.tensor_copy(sb[:, k2 : k2 + 2], st[:])
        return sb

    wq_sb = load_w(wq_d, KO, DLOC)
    wk_sb = load_w(wk_d, KO, DLOC)
    wv_sb = load_w(wv_d, KO, DLOC)

    # Wo rows regrouped as [d=64, head, H] so every head's lhsT/rhs pair for
    # the output projection lives at base partition 0.
    wo_sb = const.tile([HD, HPC, H], BF16, tag="w_wo")
    for h1 in range(HPC):
        st = stg.tile([HD, 1, H], F32, tag="wstg")
        nc.sync.dma_start(
            st[:], wo_d[:].rearrange("(h d) n -> d h n", d=HD)[:, h1 : h1 + 1]
        )
        nc.vector.tensor_copy(wo_sb[:, h1 : h1 + 1], st[:])

    # per-partition biases for Q/K projections: [DLOC] -> [P, 2]
    bq_sb = const.tile([P, DLOC // P], F32)
    nc.sync.dma_start(bq_sb[:], bq_d[:].rearrange("(o p) -> p o", p=P))
    bk_sb = const.tile([P, DLOC // P], F32)
    nc.sync.dma_start(bk_sb[:], bk_d[:].rearrange("(o p) -> p o", p=P))

    # free-axis vectors, replicated across partitions via gpsimd
    def bcast_vec(dram_t, n):
        row = stg.tile([1, n], F32, tag="wstg")
        nc.sync.dma_start(row[:], dram_t[:].rearrange("(o n) -> o n", o=1))
        bc = const.tile([P, n], F32, tag=f"bc_{dram_t.name}")
        nc.gpsimd.partition_broadcast(bc[:], row[:])
        return bc

    bv_bc = bcast_vec(bv_d, DLOC)
    bo_bc = bcast_vec(bo_d, H)
    gamma_bc = bcast_vec(gamma_d, H)
    beta_bc = bcast_vec(beta_d, H)

    eps_sb = const.tile([P, 1], F32)
    nc.vector.memset(eps_sb[:], LN_EPS)

    # ---- xT: [P, KO, S] bf16 (x transposed via PE) ----
    xT = const.tile([P, KO, S], BF16, tag="big")
    for ti in range(TI):
        xa = stg.tile([P, H], F32, tag="xstg")
        nc.sync.dma_start(xa[:], xb[ti * P : (ti + 1) * P, :])
        x16 = work.tile([P, H], BF16, tag="x16")
        nc.vector.tensor_copy(x16[:], xa[:])
        for kg in range(0, KO, 4):
            tp = psA.tile([P, QH], BF16, tag="psA")
            for j in range(4):
                nc.tensor.transpose(
                    tp[:, j * P : (j + 1) * P],
                    x16[:, (kg + j) * P : (kg + j + 1) * P],
                    ident[:],
                )
            nc.vector.tensor_copy(
                xT[:, kg : kg + 4, ti * P : (ti + 1) * P],
                tp[:, : 4 * P].rearrange("p (j t) -> p j t", j=4),
            )

    # ---- Q/K projections -> QT/KT [P, 2, S] bf16 (head-pair packed) ----
    QT = const.tile([P, DLOC // P, S], BF16)
    KT = const.tile([P, DLOC // P, S], BF16)
    for dst, w_sb, b_sb in ((QT, wq_sb, bq_sb), (KT, wk_sb, bk_sb)):
        for pr in range(DLOC // P):
            for qh in range(S // QH):
                ps = psA.tile([P, QH], F32, tag="psA")
                for ko in range(KO):
                    lhsT = w_sb[:, ko, pr * P : (pr + 1) * P]
                    for qc in range(QH // 512):
                        col = qh * QH + qc * 512
                        nc.tensor.matmul(
                            ps[:, qc * 512 : (qc + 1) * 512],
                            lhsT,
                            xT[:, ko, col : col + 512],
                            start=(ko == 0),
                            stop=(ko == KO - 1),
                        )
                nc.vector.tensor_scalar_add(
                    dst[:, pr, qh * QH : (qh + 1) * QH], ps[:], b_sb[:, pr : pr + 1]
                )

    # ---- V' [P, TI, HPC, HD+1] bf16 (ones column for exp-sums) ----
    v_sb = const.tile([P, TI, HPC, HD + 1], BF16)
    nc.vector.memset(v_sb[:, :, :, HD], 1.0)
    for ti in range(TI):
        ps = psA.tile([P, QH], F32, tag="psA")
        for ko in range(KO):
            nc.tensor.matmul(
                ps[:, :DLOC],
                xT[:, ko, ti * P : (ti + 1) * P],
                wv_sb[:, ko, :],
                start=(ko == 0),
                stop=(ko == KO - 1),
            )
        nc.vector.tensor_tensor(
            v_sb[:, ti, :, :HD],
            ps[:, :DLOC].rearrange("p (h d) -> p h d", h=HPC),
            bv_bc[:].rearrange("p (h d) -> p h d", h=HPC),
            ALU.add,
        )

    # ---- attention: per (head, q-half) ----
    ctxT = const.tile([HD, HPC, S], BF16, tag="big")
    for h in range(HPC):
        pr, off = h // 2, (h % 2) * HD
        for qh in range(S // QH):
            ctx_ps = psB.tile([P, QH], F32, tag="psB")
            for kc in range(NKC):
                ps = psA.tile([P, QH], F32, tag="psA")
                lhsT_k = KT[off : off + HD, pr, kc * P : (kc + 1) * P]
                for qc in range(QH // 512):
                    col = qh * QH + qc * 512
                    nc.tensor.matmul(
                        ps[:, qc * 512 : (qc + 1) * 512],
                        lhsT_k,
                        QT[off : off + HD, pr, col : col + 512],
                        start=True,
                        stop=True,
                    )
                ex = expp.tile([P, QH], BF16, tag="exp")
                nc.scalar.activation(ex[:], ps[:], AF.Exp, scale=1.0 / np.sqrt(HD))
                for qc in range(QH // 512):
                    nc.tensor.matmul(
                        ctx_ps[: HD + 1, qc * 512 : (qc + 1) * 512],
                        v_sb[:, kc, h, :],
                        ex[:, qc * 512 : (qc + 1) * 512],
                        start=(kc == 0),
                        stop=(kc == NKC - 1),
                    )
            # sums live on PSUM partition 64: reciprocal there (partition-
            # aligned), DMA the row to partition 0, then broadcast.
            sums = small.tile([HD + 1, QH], F32, tag="sums")
            nc.vector.reciprocal(sums[HD : HD + 1, :], ctx_ps[HD : HD + 1, :])
            rec0 = small.tile([1, QH], F32, tag="rec0")
            nc.sync.dma_start(rec0[:], sums[HD : HD + 1, :])
            recb = small.tile([HD, QH], F32, tag="recb")
            nc.gpsimd.partition_broadcast(recb[:], rec0[:])
            nc.vector.tensor_tensor(
                ctxT[:, h, qh * QH : (qh + 1) * QH],
                ctx_ps[:HD, :],
                recb[:],
                ALU.mult,
            )

    # ---- output projection -> partial [S, H] bf16 in DRAM ----
    for ti in range(TI):
        ps = psA.tile([P, QH], F32, tag="psA")
        for h in range(HPC):
            lhsT = ctxT[:, h, ti * P : (ti + 1) * P]
            woh = wo_sb[:, h, :]
            for ncn in range(H // 512):
                nc.tensor.matmul(
                    ps[:, ncn * 512 : (ncn + 1) * 512],
                    lhsT,
                    woh[:, ncn * 512 : (ncn + 1) * 512],
                    start=(h == 0),
                    stop=(h == HPC - 1),
                )
        ob = work.tile([P, H], BF16, tag="ob")
        nc.vector.tensor_copy(ob[:], ps[:])
        nc.sync.dma_start(partial_d[ti * P : (ti + 1) * P, :], ob[:])

    # ---- ReduceScatter across the 4-rank batch group ----
    nc.gpsimd.collective_compute(
        "ReduceScatter",
        ALU.add,
        replica_groups=GROUPS,
        ins=[partial_d[:].opt()],
        outs=[rs_d[:].opt()],
    )

    # ---- epilogue: residual + bias + LayerNorm on the token shard ----
    for tj in range(SSH // P):
        rs_t = epi.tile([P, H], BF16, tag="rs")
        nc.sync.dma_start(rs_t[:], rs_d[tj * P : (tj + 1) * P, :])
        xr = epi.tile([P, H], F32, tag="xr")
        nc.sync.dma_start(xr[:], xres[tj * P : (tj + 1) * P, :])
        y = epi.tile([P, H], F32, tag="y")
        nc.vector.tensor_tensor(y[:], rs_t[:], xr[:], ALU.add)
        nc.vector.tensor_tensor(y[:], y[:], bo_bc[:], ALU.add)
        stats = small.tile([P, 2, 6], F32, tag="stats")
        for sg in range(2):
            nc.vector.bn_stats(
                stats[:, sg, :], y[:].rearrange("p (s f) -> p s f", s=2)[:, sg, :]
            )
        mv = small.tile([P, 2], F32, tag="mv")
        nc.vector.bn_aggr(mv[:], stats[:])
        nc.scalar.activation(
            mv[:, 1:2], mv[:, 1:2], AF.Sqrt, bias=eps_sb[:], scale=1.0
        )
        nc.vector.reciprocal(mv[:, 1:2], mv[:, 1:2])
        nc.vector.tensor_scalar(
            out=y[:],
            in0=y[:],
            scalar1=mv[:, 0:1],
            scalar2=mv[:, 1:2],
            op0=ALU.subtract,
            op1=ALU.mult,
        )
        nc.vector.tensor_tensor(y[:], y[:], gamma_bc[:], ALU.mult)
        nc.vector.tensor_tensor(y[:], y[:], beta_bc[:], ALU.add)
        nc.sync.dma_start(out_d[tj * P : (tj + 1) * P, :], y[:])


_NC_CACHE = None


def _get_nc():
    global _NC_CACHE
    if _NC_CACHE is None:
        _NC_CACHE = build()
    return _NC_CACHE


class Runner:
    """Compile once, execute many times via PJRT (mirrors
    bass2jax.run_bass_via_pjrt but keeps the jitted executable and device
    buffers so repeated calls measure steady-state device time)."""

    def __init__(self):
        import jax
        from jax.sharding import Mesh, PartitionSpec
        from jax.experimental.shard_map import shard_map
        from concourse import bass2jax, mybir as _mb

        bass2jax.install_neuronx_cc_hook()
        nc = _get_nc()
        self.nc = nc
        partition_name = (
            nc.partition_id_tensor.name if nc.partition_id_tensor else None
        )
        in_names, out_names, out_avals, zero_outs = [], [], [], []
        for alloc in nc.m.functions[0].allocations:
            if not isinstance(alloc, _mb.MemoryLocationSet):
                continue
            name = alloc.memorylocations[0].name
            if alloc.kind == "ExternalInput":
                if name != partition_name:
                    in_names.append(name)
            elif alloc.kind == "ExternalOutput":
                shape = tuple(alloc.tensor_shape)
                dtype = _mb.dt.np(alloc.dtype)
                out_names.append(name)
                out_avals.append(jax.core.ShapedArray(shape, dtype))
                zero_outs.append(np.zeros(shape, dtype))
        self.in_names, self.out_names = in_names, out_names
        self.zero_outs = zero_outs
        n_params, n_outs = len(in_names), len(out_names)
        all_names = in_names + out_names
        if partition_name is not None:
            all_names = all_names + [partition_name]
        donate = tuple(range(n_params, n_params + n_outs))

        def _body(*args):
            operands = list(args)
            if partition_name is not None:
                operands.append(bass2jax.partition_id_tensor())
            outs = bass2jax._bass_exec_p.bind(
                *operands,
                out_avals=tuple(out_avals),
                in_names=tuple(all_names),
                out_names=tuple(out_names),
                lowering_input_output_aliases=(),
                sim_require_finite=True,
                sim_require_nnan=True,
                nc=nc,
            )
            return tuple(outs)

        devices = jax.devices()[:N_CORES]
        self.mesh = Mesh(np.asarray(devices), ("core",))
        in_specs = (PartitionSpec("core"),) * (n_params + n_outs)
        out_specs = (PartitionSpec("core"),) * n_outs
        self.sharded = jax.jit(
            shard_map(
                _body,
                mesh=self.mesh,
                in_specs=in_specs,
                out_specs=out_specs,
                check_rep=False,
            ),
            donate_argnums=donate,
            keep_unused=True,
        )
        self._jax = jax

    def device_inputs(self, in_maps):
        import jax
        from jax.sharding import NamedSharding, PartitionSpec

        sh = NamedSharding(self.mesh, PartitionSpec("core"))
        args = []
        for name in self.in_names:
            cat = np.concatenate([np.asarray(m[name]) for m in in_maps], axis=0)
            args.append(jax.device_put(cat, sh))
        outs = [
            jax.device_put(np.concatenate([z] * N_CORES, axis=0), sh)
            for z in self.zero_outs
        ]
        return args, outs

    def run(self, in_maps):
        args, outs = self.device_inputs(in_maps)
        res = self.sharded(*args, *outs)
        per_core = []
        for c in range(N_CORES):
            d = {}
            for i, name in enumerate(self.out_names):
                full = np.asarray(res[i])
                n0 = full.shape[0] // N_CORES
                d[name] = full[c * n0 : (c + 1) * n0]
            per_core.append(d)
        return per_core

    def time_exec(self, in_maps, iters=20, warmup=3):
        import time

        args, outs = self.device_inputs(in_maps)
        for _ in range(warmup):
            res = self.sharded(*args, *outs)
            outs = list(res)
        self._jax.block_until_ready(outs)
        t0 = time.perf_counter()
        for _ in range(iters):
            res = self.sharded(*args, *outs)
            outs = list(res)
        self._jax.block_until_ready(outs)
        t1 = time.perf_counter()
        return (t1 - t0) / iters


_RUNNER = None


def _get_runner():
    global _RUNNER
    if _RUNNER is None:
        _RUNNER = Runner()
    return _RUNNER


def make_in_maps(inputs):
    x = np.asarray(inputs["x"], np.float32)
    wq, wk, wv = (np.asarray(inputs[k], np.float32) for k in ("Wq", "Wk", "Wv"))
    wo = np.asarray(inputs["Wo"], np.float32)
    bq, bk, bv = (np.asarray(inputs[k], np.float32) for k in ("bq", "bk", "bv"))
    bo = np.asarray(inputs["bo"], np.float32)
    gamma = np.asarray(inputs["ln_gamma"], np.float32)
    beta = np.asarray(inputs["ln_beta"], np.float32)

    in_maps = []
    for c in range(N_CORES):
        g, r = c // RANKS, c % RANKS
        cols = slice(DLOC * r, DLOC * (r + 1))
        in_maps.append(
            {
                "xb": np.ascontiguousarray(x[g]),
                "xres": np.ascontiguousarray(x[g, SSH * r : SSH * (r + 1)]),
                "wq": np.ascontiguousarray(wq[:, cols]),
                "wk": np.ascontiguousarray(wk[:, cols]),
                "wv": np.ascontiguousarray(wv[:, cols]),
                "wo": np.ascontiguousarray(wo[cols, :]),
                "bq": np.ascontiguousarray(bq[cols]),
                "bk": np.ascontiguousarray(bk[cols]),
                "bv": np.ascontiguousarray(bv[cols]),
                "bo": bo,
                "gamma": gamma,
                "beta": beta,
            }
        )
    return in_maps


def run_spmd(inputs, trace=False):
    results = _get_runner().run(make_in_maps(inputs))
    out = np.empty((B, S, H), np.float32)
    for c in range(N_CORES):
        g, r = c // RANKS, c % RANKS
        out[g, SSH * r : SSH * (r + 1)] = results[c]["out"]
    return out, results


def kernel(**inputs) -> np.ndarray:
    out, _ = run_spmd(inputs)
    return out
